# revision 41
# baseline (speedup 1.0000x reference)
"""Distributed GATv2 kernel v2 for 8 Trainium2 NeuronCores (Bass/Tile).

Same contract as kernel.py. Redesign vs baseline:
  * Head-interleaved 256-col tables (newcol q = (h=q%4, d=q//4)), signed
    tau scaling so the whole a*leaky_relu contribution collapses into one
    Prelu activation with per-partition scale/alpha in a feature-major
    ("flipped") layout; no extra linear columns, so gather rows are
    exactly 512B (elem_size 256) instead of 768B.
  * u gathered twice per edge (edge-major for output weighting, flipped
    for scores) via dma_gather transpose mode; v is never gathered: a
    per-bin V block (slot-major, host-reordered) is combined through a
    selection-matrix matmul on the TensorEngine (v-select), which also
    adds u via an identity matmul into the same PSUM accumulation.
  * Scores come from tiny per-tile matmuls against a head-selector
    (contraction over feature partitions), killing the vector-engine
    reduce; exp writes directly into the aggregation operand's last 4
    columns, giving softmax denominators through the same selection
    matmul as the features.
  * Projections run in bf16 (4x faster PE, half the DMA).
"""
import os
import sys

for _p in ("/opt/trn_rl_repo",):
    if _p not in sys.path:
        sys.path.insert(0, _p)

from contextlib import ExitStack

import numpy as np
import ml_dtypes

import concourse.bass as bass
import concourse.tile as tile
from concourse import bacc, bass_utils, mybir
from concourse._compat import with_exitstack

F32 = mybir.dt.float32
BF16 = mybir.dt.bfloat16
I16 = mybir.dt.int16
P = 128
N_CORES = 8
BIN_DSTS = 125
TRASH_SLOT = 127
NEG_SLOPE = 0.2
GW = 4
BF = ml_dtypes.bfloat16

# ---------------------------------------------------------------- host prep


def _fold_weights(W_src, b_src, W_dst, b_dst, attn):
    H, DO = attn.shape
    D = H * DO
    a = attn.reshape(-1).astype(np.float64)       # orig col c = h*DO + d
    assert np.abs(a).min() > 1e-8, "attn coefficient too close to zero"
    q = np.arange(D)
    c_of_q = (q % H) * DO + q // H                # newcol -> orig col
    aq = a[c_of_q]
    tau = np.where(aq > 0, aq, -NEG_SLOPE * aq)   # >0
    sigma = np.where(aq > 0, 1.0, -1.0)
    alpha = np.where(aq > 0, NEG_SLOPE, 1.0 / NEG_SLOPE)

    Ws = W_src.astype(np.float64)
    Wd = W_dst.astype(np.float64)
    bsum = (b_src.astype(np.float64) + b_dst.astype(np.float64))
    WsP = (tau[None, :] * Ws[c_of_q, :].T).astype(np.float32)   # [D_in, D]
    WdP = (tau[None, :] * Wd[c_of_q, :].T).astype(np.float32)
    bV = (tau * bsum[c_of_q]).astype(np.float32)                # [D]

    INVA = np.broadcast_to((1.0 / tau).astype(np.float32), (P, D)).copy()
    SCALE = sigma.reshape(2, P).T.astype(np.float32).copy()     # [P, 2]
    ALPHA = alpha.reshape(2, P).T.astype(np.float32).copy()
    return WsP, WdP, bV, INVA, SCALE, ALPHA, c_of_q


def _build_plan(src, dst, n_nodes, n_cores, chunk_rows):
    E = src.shape[0]
    deg = np.bincount(dst, minlength=n_nodes).astype(np.int64)
    degA = np.bincount(dst[src < chunk_rows], minlength=n_nodes).astype(
        np.int64)
    csum = np.cumsum(deg)
    bounds = [0]
    for c in range(1, n_cores):
        bounds.append(int(np.searchsorted(csum, E * c // n_cores)))
    bounds.append(n_nodes)
    order = np.argsort(dst, kind="stable")
    dst_sorted = dst[order]
    lo = np.searchsorted(dst_sorted, np.array(bounds[:-1]))
    hi = np.searchsorted(dst_sorted, np.array(bounds[1:]))

    max_nlocal = max(bounds[c + 1] - bounds[c] for c in range(n_cores))
    B = int(np.ceil(max_nlocal / BIN_DSTS))
    core_bins = []
    for c in range(n_cores):
        d0, d1 = bounds[c], bounds[c + 1]
        dA = degA[d0:d1]
        dB = deg[d0:d1] - dA
        od = np.argsort(-(dA + dB), kind="stable")
        binof = np.empty(d1 - d0, np.int32)
        slotof = np.empty(d1 - d0, np.int32)
        loadA = np.zeros(B)
        loadB = np.zeros(B)
        cnt = np.zeros(B, np.int64)
        for i in od:
            full = cnt >= BIN_DSTS
            costs = np.maximum(loadA + dA[i], loadB + dB[i]) + full * 1e12
            bsel = int(np.argmin(costs))
            binof[i] = bsel
            slotof[i] = cnt[bsel]
            cnt[bsel] += 1
            loadA[bsel] += dA[i]
            loadB[bsel] += dB[i]
        assert slotof.max(initial=0) < TRASH_SLOT
        core_bins.append((binof, slotof))
    return {"bounds": bounds, "order": order, "lo": lo, "hi": hi, "B": B,
            "core_bins": core_bins}


def _core_chunk_counts(plan, src, dst, core, chunk_rows):
    d0 = plan["bounds"][core]
    binof, _ = plan["core_bins"][core]
    eidx = plan["order"][plan["lo"][core]:plan["hi"][core]]
    esrc = src[eidx].astype(np.int64)
    eloc = dst[eidx].astype(np.int64) - d0
    eb = binof[eloc]
    B = plan["B"]
    ca = np.bincount(eb[esrc < chunk_rows], minlength=B)
    cb = np.bincount(eb[esrc >= chunk_rows], minlength=B)
    return ca, cb


def _wrap16(flat):
    # [B, n] int16 -> [128, B * n/16] wrapped in 16 partitions, tiled
    Bn, n = flat.shape
    w = flat.reshape(Bn, n // 16, 16).transpose(0, 2, 1)      # [B,16,c]
    w = np.tile(w, (1, 8, 1))                                 # [B,128,c]
    return np.ascontiguousarray(w.transpose(1, 0, 2).reshape(P, -1))


def _build_core_arrays(plan, src, dst, core, n_nodes, chunk_rows, TA, TB):
    B = plan["B"]
    T = TA + TB
    d0 = plan["bounds"][core]
    d1 = plan["bounds"][core + 1]
    binof, slotof = plan["core_bins"][core]
    eidx = plan["order"][plan["lo"][core]:plan["hi"][core]]
    esrc = src[eidx].astype(np.int64)
    eloc = dst[eidx].astype(np.int64) - d0
    eb = binof[eloc]
    chunk = (esrc >= chunk_rows).astype(np.int64)
    o2 = np.lexsort((esrc, chunk, eb))
    eb, esrc_o, eloc_o, ch_o = eb[o2], esrc[o2], eloc[o2], chunk[o2]
    eslot = slotof[eloc_o]
    nE = len(eidx)
    key = eb * 2 + ch_o
    gstart = np.searchsorted(key, np.arange(2 * B))
    rank = np.arange(nE) - gstart[key]
    pos = np.where(ch_o == 0, rank, rank + TA * P)
    t, pp = np.divmod(pos, P)
    assert (np.where(ch_o == 0, t, 0)).max(initial=0) < TA
    assert t.max(initial=0) < T
    colpos = eb * T + t

    dstloc = np.full((P, B * T), TRASH_SLOT, np.float32)
    dstloc[pp, colpos] = eslot
    # transposed copy: per (bin,tile) column-major edge order along free dim
    dstlocT = np.ascontiguousarray(
        dstloc.T.reshape(1, B * T * P)).astype(BF)

    flat = np.zeros((B, T * P), np.int16)
    selA = ch_o == 0
    flat[eb[selA], pos[selA]] = esrc_o[selA].astype(np.int16)
    selB = ch_o == 1
    flat[eb[selB], pos[selB]] = (esrc_o[selB]
                                 - chunk_rows).astype(np.int16)
    idxAB = _wrap16(flat)

    outidx = np.full((P, B), n_nodes, np.int32)
    outidx[slotof, binof] = np.arange(d1 - d0) + d0
    return {"idxAB": idxAB, "dstloc": dstloc,
            "dstlocT": dstlocT, "outidx": outidx,
            "binof": binof, "slotof": slotof, "d0": d0, "d1": d1}


def _prepare(inputs, n_cores):
    feat = np.ascontiguousarray(np.asarray(inputs["feat"], np.float32))
    src = np.asarray(inputs["src"])
    dst = np.asarray(inputs["dst"])
    attn = np.asarray(inputs["attn"], np.float32)
    N, D = feat.shape
    H = attn.shape[0]

    WsP, WdP, bV, INVA, SCALE, ALPHA, c_of_q = _fold_weights(
        inputs["W_src"], inputs["b_src"], inputs["W_dst"], inputs["b_dst"],
        attn)
    NPAD = ((N + 2 * P - 1) // (2 * P)) * (2 * P)
    CHK = NPAD // 2
    assert CHK < 32768, "chunk too large for int16 dma_gather indices"
    plan = _build_plan(src, dst, N, n_cores, CHK)
    B = plan["B"]
    TA = TB = 0
    for c in range(n_cores):
        ca, cb = _core_chunk_counts(plan, src, dst, c, CHK)
        TA = max(TA, int(np.ceil(ca.max() / P)))
        TB = max(TB, int(np.ceil(cb.max() / P)))
    TA = ((TA + GW - 1) // GW) * GW
    TB = ((TB + GW - 1) // GW) * GW
    T = TA + TB
    VROWS = B * P

    featT = np.zeros((D, NPAD), np.float32)
    featT[:, :N] = feat.T
    featT_bf = featT.astype(BF)
    IOTA = np.ascontiguousarray(np.broadcast_to(
        np.arange(P, dtype=np.float32), (P, GW, P))).astype(BF)
    IOTAP = np.arange(P, dtype=np.float32).reshape(P, 1).astype(BF)
    ONES1 = np.ones((1, P), np.float32).astype(BF)
    IDENT = np.eye(P, dtype=np.float32).astype(BF)
    HSEL = np.zeros((P, H), np.float32)
    HSEL[np.arange(P), np.arange(P) % H] = 1.0
    HSEL = HSEL.astype(BF)

    cfg = {"N": N, "NPAD": NPAD, "B": B, "T": T, "D": D,
           "VROWS": VROWS, "H": H, "TA": TA, "TB": TB, "CHK": CHK}
    WsP_r = np.ascontiguousarray(
        WsP.reshape(2, P, D)).astype(BF)                      # [2, P, D]
    WdP_r = np.ascontiguousarray(WdP.reshape(2, P, D)).astype(BF)
    bV_r = bV.reshape(1, D).astype(BF)

    in_maps = []
    coreinfo = []
    for c in range(n_cores):
        arrs = _build_core_arrays(plan, src, dst, c, N, CHK, TA, TB)
        d0, d1 = arrs["d0"], arrs["d1"]
        binof, slotof = arrs["binof"], arrs["slotof"]
        featTd = np.zeros((D + 1, VROWS), np.float32)
        cols = binof.astype(np.int64) * P + slotof.astype(np.int64)
        featTd[:D, cols] = featT[:, d0 + np.arange(d1 - d0)]
        featTd[D, cols] = 1.0
        in_maps.append({
            "featT": featT_bf, "featTd": featTd.astype(BF),
            "WsP": WsP_r, "WdP": WdP_r, "bV": bV_r,
            "INVA": INVA, "SCALE": SCALE, "ALPHA": ALPHA,
            "IOTA": IOTA, "IOTAP": IOTAP, "ONES1": ONES1,
            "IDENT": IDENT, "HSEL": HSEL,
            "idxAB": arrs["idxAB"],
            "dstloc": arrs["dstloc"], "dstlocT": arrs["dstlocT"],
        })
        coreinfo.append((d0, d1, arrs["outidx"]))
    return cfg, in_maps, coreinfo


# ------------------------------------------------------------- bass program


def build_program(cfg, num_devices, repeat=1):
    N, NPAD, B, T = cfg["N"], cfg["NPAD"], cfg["B"], cfg["T"]
    D, VROWS, H = cfg["D"], cfg["VROWS"], cfg["H"]
    TA, TB = cfg["TA"], cfg["TB"]
    assert NPAD % P == 0

    _nq = int(os.environ.get("GAT_QUEUES", "4"))
    nc = bacc.Bacc("TRN2", target_bir_lowering=False, debug=False,
                   num_devices=num_devices,
                   num_swdge_queues=_nq,
                   dynamic_dma_scratch_size=int(
                       os.environ.get("GAT_SCRATCH", "16384")))
    if _nq > 1 and os.environ.get("GAT_NQATTR", "0") == "1":
        nc.m.attributes = (nc.m.attributes or {}) | {
            "num_swdge_queues": _nq}

    featT = nc.dram_tensor("featT", (D, NPAD), BF16, kind="ExternalInput")
    featTd = nc.dram_tensor("featTd", (D + 1, VROWS), BF16,
                            kind="ExternalInput")
    WsP = nc.dram_tensor("WsP", (2, P, D), BF16, kind="ExternalInput")
    WdP = nc.dram_tensor("WdP", (2, P, D), BF16, kind="ExternalInput")
    bV = nc.dram_tensor("bV", (1, D), BF16, kind="ExternalInput")
    INVA = None
    SCALE = nc.dram_tensor("SCALE", (P, 2), F32, kind="ExternalInput")
    ALPHA = nc.dram_tensor("ALPHA", (P, 2), F32, kind="ExternalInput")
    IOTA = nc.dram_tensor("IOTA", (P, GW * P), BF16, kind="ExternalInput")
    IOTAP = nc.dram_tensor("IOTAP", (P, 1), BF16, kind="ExternalInput")
    ONES1 = nc.dram_tensor("ONES1", (1, P), BF16, kind="ExternalInput")
    IDENT = nc.dram_tensor("IDENT", (P, P), BF16, kind="ExternalInput")
    HSEL = nc.dram_tensor("HSEL", (P, H), BF16, kind="ExternalInput")
    idxAB = nc.dram_tensor("idxAB", (P, B * T * P // 16), I16,
                           kind="ExternalInput")
    dstloc = nc.dram_tensor("dstloc", (P, B * T), F32, kind="ExternalInput")
    dstlocT = nc.dram_tensor("dstlocT", (1, B * T * P), BF16,
                             kind="ExternalInput")
    OUT = nc.dram_tensor("OUT", (B * P, D + H), BF16,
                         kind="ExternalOutput")
    Uext = [nc.dram_tensor("UextA", (NPAD // 2, D), BF16, kind="Internal"),
            nc.dram_tensor("UextB", (NPAD // 2, D), BF16, kind="Internal")]
    Vext = nc.dram_tensor("Vext", (VROWS, D), BF16, kind="Internal")

    with tile.TileContext(nc) as tc:
        _build_body(tc, nc, cfg, featT, featTd, WsP, WdP, bV, INVA, SCALE,
                    ALPHA, IOTA, IOTAP, ONES1, IDENT, HSEL, idxAB,
                    dstloc, dstlocT, OUT, Uext, Vext, repeat=repeat)
    nc.compile()
    return nc


@with_exitstack
def _build_body(ctx: ExitStack, tc, nc, cfg, featT, featTd, WsP, WdP, bV,
                INVA, SCALE, ALPHA, IOTA, IOTAP, ONES1, IDENT, HSEL,
                idxAB, dstloc, dstlocT, OUT, Uext, Vext, repeat=1):
    N, NPAD, B, T = cfg["N"], cfg["NPAD"], cfg["B"], cfg["T"]
    D, VROWS, H = cfg["D"], cfg["VROWS"], cfg["H"]
    TA, TB, CHK = cfg["TA"], cfg["TB"], cfg["CHK"]

    cpool = ctx.enter_context(tc.tile_pool(name="consts", bufs=1))

    # SWDGE queue assignment: the tile framework round-robins Pool-engine
    # DMAs onto 8 completion-sem lanes in issue order; completions within a
    # lane must stay in-order, so pin queue = issue_index % nq (8 % nq == 0
    # keeps each lane on a single queue).
    _qctr = [0]

    def next_q(nq):
        v = _qctr[0] % nq
        _qctr[0] += 1
        return v

    ws = cpool.tile([P, 2, D], BF16)
    nc.sync.dma_start(ws[:], WsP[:].rearrange("a p x -> p a x"))
    wd = cpool.tile([P, 2, D], BF16)
    nc.sync.dma_start(wd[:], WdP[:].rearrange("a p x -> p a x"))
    wdb = cpool.tile([1, D], BF16)
    nc.sync.dma_start(wdb[:], bV[:])
    scl = cpool.tile([P, 2], F32)
    nc.sync.dma_start(scl[:], SCALE[:])
    alp = cpool.tile([P, 2], F32)
    nc.sync.dma_start(alp[:], ALPHA[:])
    iota = cpool.tile([P, GW, P], BF16)
    nc.sync.dma_start(iota[:], IOTA[:].rearrange("p (g i) -> p g i", g=GW))
    iotap = cpool.tile([P, 1], BF16)
    nc.sync.dma_start(iotap[:], IOTAP[:])
    ones1 = cpool.tile([1, P], BF16)
    nc.sync.dma_start(ones1[:], ONES1[:])
    ident = cpool.tile([P, P], BF16)
    nc.sync.dma_start(ident[:], IDENT[:])
    hsel = cpool.tile([P, H], BF16)
    nc.sync.dma_start(hsel[:], HSEL[:])
    dlc = cpool.tile([P, B * T], F32)
    nc.sync.dma_start(dlc[:], dstloc[:])

    phase = os.environ.get("GAT_PHASE", "full")

    def env(k, d):
        return int(os.environ.get(k, str(d)))

    fpool = ctx.enter_context(tc.tile_pool(name="proj", bufs=3))
    upool = ctx.enter_context(tc.tile_pool(name="uout", bufs=4))
    pj = ctx.enter_context(tc.tile_pool(name="psum_proj",
                                        bufs=env("GAT_PJ", 2),
                                        space="PSUM"))
    epool = ctx.enter_context(tc.tile_pool(name="edge", bufs=2))
    spool = ctx.enter_context(tc.tile_pool(name="small", bufs=2))
    pqv = ctx.enter_context(tc.tile_pool(name="psum_qv",
                                         bufs=env("GAT_QV", 2),
                                         space="PSUM"))
    pmisc = ctx.enter_context(tc.tile_pool(name="psum_misc", bufs=1,
                                           space="PSUM"))
    pa = ctx.enter_context(tc.tile_pool(name="psum_agg",
                                        bufs=env("GAT_PAGG", 2),
                                        space="PSUM"))

    for rep in range(repeat):
      if True:

        vball = epool.tile([P, B, D], BF16, tag="vball", bufs=1)

        def projectV():
            CH = int(os.environ.get("GAT_CH", "1024"))
            nchunks = (VROWS + CH - 1) // CH
            for ci in range(nchunks):
                c0 = ci * CH
                cw = min(CH, VROWS - c0)
                nsub = cw // P
                ft = fpool.tile([P, 2, CH], BF16, tag="ft")
                nc.sync.dma_start(
                    ft[:, :, :cw],
                    featTd[0:D, :].rearrange("(a p) n -> p a n",
                                             p=P)[:, :, c0:c0 + cw])
                ftb = fpool.tile([1, CH], BF16, tag="ftb", name="ftb")
                nc.sync.dma_start(ftb[:, :cw], featTd[D:D + 1, c0:c0 + cw])
                for pi in range(0, nsub, 2):
                    pw = min(2, nsub - pi)
                    ps = pj.tile([P, 2, D], F32, tag="pv")
                    for k in range(pw):
                        si = (pi + k) * P
                        nc.tensor.matmul(ps[:, k], lhsT=ft[:, 0, si:si + P],
                                         rhs=wd[:, 0], start=True,
                                         stop=False)
                        nc.tensor.matmul(ps[:, k], lhsT=ft[:, 1, si:si + P],
                                         rhs=wd[:, 1], start=False,
                                         stop=False)
                        nc.tensor.matmul(ps[:, k], lhsT=ftb[:, si:si + P],
                                         rhs=wdb[:], start=False, stop=True)
                    if (pi // 2) % 2 == 0:
                        nc.scalar.copy(
                            vball[:, c0 // P + pi:c0 // P + pi + pw],
                            ps[:, :pw])
                    else:
                        nc.vector.tensor_copy(
                            vball[:, c0 // P + pi:c0 // P + pi + pw],
                            ps[:, :pw])

        def project(ft_dram, ncols, wtile, wbias, table):
            CH = int(os.environ.get("GAT_CH", "1024"))
            assert ncols % P == 0
            nchunks = (ncols + CH - 1) // CH
            for ci in range(nchunks):
                c0 = ci * CH
                cw = min(CH, ncols - c0)
                nsub = cw // P
                ft = fpool.tile([P, 2, CH], BF16, tag="ft")
                nc.sync.dma_start(
                    ft[:, :, :cw],
                    ft_dram[0:D, :].rearrange("(a p) n -> p a n",
                                              p=P)[:, :, c0:c0 + cw])
                if wbias is not None:
                    ftb = fpool.tile([1, CH], BF16, tag="ftb", name="ftb")
                    nc.sync.dma_start(ftb[:, :cw],
                                      ft_dram[D:D + 1, c0:c0 + cw])
                ub = upool.tile([P, CH // P, D], BF16)
                for pi in range(0, nsub, 2):
                    pw = min(2, nsub - pi)
                    ps = pj.tile([P, 2, D], F32, tag="pv")
                    for k in range(pw):
                        si = (pi + k) * P
                        nc.tensor.matmul(ps[:, k], lhsT=ft[:, 0, si:si + P],
                                         rhs=wtile[:, 0], start=True,
                                         stop=False)
                        nc.tensor.matmul(ps[:, k], lhsT=ft[:, 1, si:si + P],
                                         rhs=wtile[:, 1], start=False,
                                         stop=wbias is None)
                        if wbias is not None:
                            nc.tensor.matmul(ps[:, k],
                                             lhsT=ftb[:, si:si + P],
                                             rhs=wbias[:], start=False,
                                             stop=True)
                    if (pi // 2) % 2 == 0:
                        nc.scalar.copy(ub[:, pi:pi + pw], ps[:, :pw])
                    else:
                        nc.vector.tensor_copy(ub[:, pi:pi + pw],
                                              ps[:, :pw])
                nc.sync.dma_start(
                    table[c0:c0 + cw, :].rearrange("(s p) d -> p s d", p=P),
                    ub[:, :nsub])

        project(featT[:, 0:CHK], CHK, ws, None, Uext[0])
        projectV()
        project(featT[:, CHK:2 * CHK], CHK, ws, None, Uext[1])

      if phase == "proj":
          continue

      nq = int(os.environ.get("GAT_QUEUES", "4"))

      if phase in ("gatherbig", "gathert", "gathertbig", "gatherq"):
        for b in range(B):
          ix = spool.tile([P, T * P // 16], I16, tag="ix", bufs=3)
          nc.sync.dma_start(ix[:], idxAB[:, b * T * P // 16:
                                         (b + 1) * T * P // 16])
          sink = spool.tile([P, 1], BF16, tag="sink", name="sink")
          if phase == "gatherbig":
              ug = epool.tile([P, T, D], BF16, tag="ug", bufs=3)
              nc.gpsimd.dma_gather(ug[:, :TA], Uext[0][:],
                                   ix[:, :TA * P // 16], TA * P, TA * P, D)
              nc.vector.tensor_copy(sink[:], ug[:, 0, 0:1])
          elif phase == "gatherq":
              ug = epool.tile([P, T, D], BF16, tag="ug", bufs=3)
              TQ = T // 4
              for k in range(4):
                  nc.gpsimd.dma_gather(
                      ug[:, k * TQ:(k + 1) * TQ], Uext[0][:],
                      ix[:, k * TQ * P // 16:(k + 1) * TQ * P // 16],
                      TQ * P, TQ * P, D, queue_num=k % nq)
              nc.vector.tensor_copy(sink[:], ug[:, 0, 0:1])
          elif phase == "gathert":
              for gi in range(T // GW):
                  gt = gi * GW
                  uf = epool.tile([P, 2, GW * P], BF16, tag="uf",
                                  name="uf", bufs=5)
                  base = Uext[0][:] if gt < TA else Uext[1][:]
                  nc.gpsimd.dma_gather(
                      uf[:], base, ix[:, gt * P // 16:(gt + GW) * P // 16],
                      GW * P, GW * P, D, transpose=True,
                      queue_num=gi % nq)
              nc.vector.tensor_copy(sink[:], uf[:, 0, 0:1])
          else:
              ufb = epool.tile([P, 2, T * P // 2], BF16, tag="ufb",
                               bufs=2)
              nc.gpsimd.dma_gather(ufb[:], Uext[0][:],
                                   ix[:, :T * P // 32], T * P // 2,
                                   T * P // 2, D, transpose=True)
              nc.vector.tensor_copy(sink[:], ufb[:, 0, 0:1])
        continue

      if True:
       for b in range(B):
        ix = spool.tile([P, T * P // 16], I16, tag="ix", bufs=4)
        nc.sync.dma_start(ix[:], idxAB[:, b * T * P // 16:
                                       (b + 1) * T * P // 16])
        vb = vball[:, b]

        ug = epool.tile([P, T, D], BF16, tag="ug", bufs=4)
        TH = TA // 2
        for hf in range(2):
            nc.gpsimd.dma_gather(
                ug[:, hf * TH:(hf + 1) * TH], Uext[0][:],
                ix[:, hf * TH * P // 16:(hf + 1) * TH * P // 16],
                TH * P, TH * P, D, queue_num=next_q(nq))
        for hf in range(2):
            nc.gpsimd.dma_gather(
                ug[:, TA + hf * TH:TA + (hf + 1) * TH], Uext[1][:],
                ix[:, (TA + hf * TH) * P // 16:
                      (TA + (hf + 1) * TH) * P // 16],
                TH * P, TH * P, D, queue_num=next_q(nq))
        dlt = spool.tile([1, T * P], BF16, tag="dlt", bufs=4)
        nc.sync.dma_start(dlt[:], dstlocT[:, b * T * P:(b + 1) * T * P])

        pagg = pa.tile([P, D + H], F32, tag="pagg")
        ngroups = T // GW
        for gi in range(ngroups):
            gt = gi * GW
            col = b * T + gt
            uf = epool.tile([P, 2, GW * P], BF16, tag="uf", name="uf",
                            bufs=6)
            base = Uext[0][:] if gt < TA else Uext[1][:]
            nc.gpsimd.dma_gather(
                uf[:], base, ix[:, gt * P // 16:(gt + GW) * P // 16],
                GW * P, GW * P, D, transpose=True,
                queue_num=next_q(nq))

            s4g = epool.tile([P, GW, P], BF16, tag="s4g", name="s4g",
                             bufs=4)
            nc.vector.tensor_tensor(
                out=s4g[:], in0=iota[:],
                in1=dlc[:, col:col + GW, None].to_broadcast([P, GW, P]),
                op=mybir.AluOpType.is_equal)
            repl = pmisc.tile([P, GW * P], F32, tag="repl",
                              bufs=env("GAT_REPL", 1))
            nc.tensor.matmul(repl[:], lhsT=ones1[:],
                             rhs=dlt[:, gt * P:(gt + GW) * P],
                             start=True, stop=True)
            s4t = epool.tile([P, GW * P], BF16, tag="s4t", name="s4t",
                             bufs=4)
            nc.vector.tensor_tensor(
                out=s4t[:], in0=iotap[:].to_broadcast([P, GW * P]),
                in1=repl[:], op=mybir.AluOpType.is_equal)

            rgT = epool.tile([P, 2, GW * P], BF16, tag="rgT", name="rgT",
                             bufs=4)
            for j in range(2):
                qv = pqv.tile([P, GW * P], F32, tag="qv")
                nc.tensor.matmul(qv[:], lhsT=vb[:, j * P:(j + 1) * P],
                                 rhs=s4t[:], start=True, stop=False)
                nc.tensor.matmul(qv[:], lhsT=ident[:], rhs=uf[:, j],
                                 start=False, stop=True)
                nc.scalar.activation(rgT[:, j], qv[:],
                                     mybir.ActivationFunctionType.Prelu,
                                     scale=scl[:, j:j + 1],
                                     alpha=alp[:, j:j + 1])

            psc = pmisc.tile([P, GW, H], F32, tag="psc",
                             bufs=env("GAT_PSC", 1))
            for t in range(GW):
                nc.tensor.matmul(psc[:, t],
                                 lhsT=rgT[:, 0, t * P:(t + 1) * P],
                                 rhs=hsel[:], start=True, stop=False)
                nc.tensor.matmul(psc[:, t],
                                 lhsT=rgT[:, 1, t * P:(t + 1) * P],
                                 rhs=hsel[:], start=False, stop=True)

            wg = epool.tile([P, GW, D + H], BF16, tag="wg", name="wg",
                            bufs=4)
            expv = wg[:, :, D:D + H]
            nc.scalar.activation(expv, psc[:],
                                 mybir.ActivationFunctionType.Exp)
            nc.vector.tensor_tensor(
                out=wg[:, :, :D].rearrange("p g (c h) -> p g c h", h=H),
                in0=ug[:, gt:gt + GW].rearrange("p g (c h) -> p g c h", h=H),
                in1=wg[:, :, None, D:D + H].to_broadcast([P, GW, D // H, H]),
                op=mybir.AluOpType.mult)
            for t in range(GW):
                nc.tensor.matmul(pagg[:], lhsT=s4g[:, t], rhs=wg[:, t],
                                 start=(gi == 0 and t == 0),
                                 stop=(gi == ngroups - 1 and t == GW - 1))

        o2 = spool.tile([P, D + H], BF16, tag="o2")
        nc.scalar.copy(o2[:], pagg[:])
        nc.sync.dma_start(OUT[b * P:(b + 1) * P, :], o2[:])


# ------------------------------------------------------------------ entry

_LAST_RESULTS = {}


def kernel(**inputs):
    import time as _time
    t0 = _time.time()
    cfg, in_maps, coreinfo = _prepare(inputs, N_CORES)
    _LAST_RESULTS["inva_row"] = np.asarray(in_maps[0]["INVA"][0],
                                           np.float32)
    t1 = _time.time()
    nc = build_program(cfg, N_CORES,
                       repeat=int(os.environ.get("GAT_REPEAT", "1")))
    t2 = _time.time()
    res = bass_utils.run_bass_kernel_spmd(
        nc, in_maps, core_ids=list(range(N_CORES)))
    t3 = _time.time()
    print(f"[kernel] prep {t1-t0:.1f}s build+compile {t2-t1:.1f}s "
          f"run {t3-t2:.1f}s", file=sys.stderr)
    _LAST_RESULTS["res"] = res
    _LAST_RESULTS["cfg"] = cfg
    N, D, H = cfg["N"], cfg["D"], cfg["H"]
    DO = D // H
    inva_row = _LAST_RESULTS.get("inva_row")
    out = np.zeros((N, D), np.float32)
    for c, (d0, d1, outidx) in enumerate(coreinfo):
        gg = outidx.T.reshape(-1)
        mask = gg < N
        raw = np.asarray(res.results[c]["OUT"], np.float32)
        den = raw[:, D:D + H]
        den = np.where(den == 0, 1.0, den)
        norm = (raw[:, :D].reshape(-1, DO, H) / den[:, None, :]
                ).reshape(-1, D) * inva_row[None, :]
        out[gg[mask]] = norm[mask]
    # newcol q -> (h = q % H, d = q // H); undo on host + u-side bias
    out = out.reshape(N, DO, H).transpose(0, 2, 1)
    b_src = np.asarray(inputs["b_src"], np.float32)
    if np.any(b_src):
        out = out + b_src.reshape(1, H, DO)
    return np.ascontiguousarray(out)



# revision 42
# speedup vs baseline: 1.0148x; 1.0148x over previous
"""Distributed GATv2 kernel v2 for 8 Trainium2 NeuronCores (Bass/Tile).

Same contract as kernel.py. Redesign vs baseline:
  * Head-interleaved 256-col tables (newcol q = (h=q%4, d=q//4)), signed
    tau scaling so the whole a*leaky_relu contribution collapses into one
    Prelu activation with per-partition scale/alpha in a feature-major
    ("flipped") layout; no extra linear columns, so gather rows are
    exactly 512B (elem_size 256) instead of 768B.
  * u gathered twice per edge (edge-major for output weighting, flipped
    for scores) via dma_gather transpose mode; v is never gathered: a
    per-bin V block (slot-major, host-reordered) is combined through a
    selection-matrix matmul on the TensorEngine (v-select), which also
    adds u via an identity matmul into the same PSUM accumulation.
  * Scores come from tiny per-tile matmuls against a head-selector
    (contraction over feature partitions), killing the vector-engine
    reduce; exp writes directly into the aggregation operand's last 4
    columns, giving softmax denominators through the same selection
    matmul as the features.
  * Projections run in bf16 (4x faster PE, half the DMA).
"""
import os
import sys

for _p in ("/opt/trn_rl_repo",):
    if _p not in sys.path:
        sys.path.insert(0, _p)

from contextlib import ExitStack

import numpy as np
import ml_dtypes

import concourse.bass as bass
import concourse.tile as tile
from concourse import bacc, bass_utils, mybir
from concourse._compat import with_exitstack

F32 = mybir.dt.float32
BF16 = mybir.dt.bfloat16
I16 = mybir.dt.int16
P = 128
N_CORES = 8
BIN_DSTS = 125
TRASH_SLOT = 127
NEG_SLOPE = 0.2
GW = 4
BF = ml_dtypes.bfloat16

# ---------------------------------------------------------------- host prep


def _fold_weights(W_src, b_src, W_dst, b_dst, attn):
    H, DO = attn.shape
    D = H * DO
    a = attn.reshape(-1).astype(np.float64)       # orig col c = h*DO + d
    assert np.abs(a).min() > 1e-8, "attn coefficient too close to zero"
    q = np.arange(D)
    c_of_q = (q % H) * DO + q // H                # newcol -> orig col
    aq = a[c_of_q]
    tau = np.where(aq > 0, aq, -NEG_SLOPE * aq)   # >0
    sigma = np.where(aq > 0, 1.0, -1.0)
    alpha = np.where(aq > 0, NEG_SLOPE, 1.0 / NEG_SLOPE)

    Ws = W_src.astype(np.float64)
    Wd = W_dst.astype(np.float64)
    bsum = (b_src.astype(np.float64) + b_dst.astype(np.float64))
    WsP = (tau[None, :] * Ws[c_of_q, :].T).astype(np.float32)   # [D_in, D]
    WdP = (tau[None, :] * Wd[c_of_q, :].T).astype(np.float32)
    bV = (tau * bsum[c_of_q]).astype(np.float32)                # [D]

    INVA = np.broadcast_to((1.0 / tau).astype(np.float32), (P, D)).copy()
    SCALE = sigma.reshape(2, P).T.astype(np.float32).copy()     # [P, 2]
    ALPHA = alpha.reshape(2, P).T.astype(np.float32).copy()
    return WsP, WdP, bV, INVA, SCALE, ALPHA, c_of_q


def _build_plan(src, dst, n_nodes, n_cores, chunk_rows):
    E = src.shape[0]
    deg = np.bincount(dst, minlength=n_nodes).astype(np.int64)
    degA = np.bincount(dst[src < chunk_rows], minlength=n_nodes).astype(
        np.int64)
    csum = np.cumsum(deg)
    bounds = [0]
    for c in range(1, n_cores):
        bounds.append(int(np.searchsorted(csum, E * c // n_cores)))
    bounds.append(n_nodes)
    order = np.argsort(dst, kind="stable")
    dst_sorted = dst[order]
    lo = np.searchsorted(dst_sorted, np.array(bounds[:-1]))
    hi = np.searchsorted(dst_sorted, np.array(bounds[1:]))

    max_nlocal = max(bounds[c + 1] - bounds[c] for c in range(n_cores))
    B = int(np.ceil(max_nlocal / BIN_DSTS))
    core_bins = []
    for c in range(n_cores):
        d0, d1 = bounds[c], bounds[c + 1]
        dA = degA[d0:d1]
        dB = deg[d0:d1] - dA
        od = np.argsort(-(dA + dB), kind="stable")
        binof = np.empty(d1 - d0, np.int32)
        slotof = np.empty(d1 - d0, np.int32)
        loadA = np.zeros(B)
        loadB = np.zeros(B)
        cnt = np.zeros(B, np.int64)
        for i in od:
            full = cnt >= BIN_DSTS
            costs = np.maximum(loadA + dA[i], loadB + dB[i]) + full * 1e12
            bsel = int(np.argmin(costs))
            binof[i] = bsel
            slotof[i] = cnt[bsel]
            cnt[bsel] += 1
            loadA[bsel] += dA[i]
            loadB[bsel] += dB[i]
        assert slotof.max(initial=0) < TRASH_SLOT
        core_bins.append((binof, slotof))
    return {"bounds": bounds, "order": order, "lo": lo, "hi": hi, "B": B,
            "core_bins": core_bins}


def _core_chunk_counts(plan, src, dst, core, chunk_rows):
    d0 = plan["bounds"][core]
    binof, _ = plan["core_bins"][core]
    eidx = plan["order"][plan["lo"][core]:plan["hi"][core]]
    esrc = src[eidx].astype(np.int64)
    eloc = dst[eidx].astype(np.int64) - d0
    eb = binof[eloc]
    B = plan["B"]
    ca = np.bincount(eb[esrc < chunk_rows], minlength=B)
    cb = np.bincount(eb[esrc >= chunk_rows], minlength=B)
    return ca, cb


def _wrap16(flat):
    # [B, n] int16 -> [128, B * n/16] wrapped in 16 partitions, tiled
    Bn, n = flat.shape
    w = flat.reshape(Bn, n // 16, 16).transpose(0, 2, 1)      # [B,16,c]
    w = np.tile(w, (1, 8, 1))                                 # [B,128,c]
    return np.ascontiguousarray(w.transpose(1, 0, 2).reshape(P, -1))


def _build_core_arrays(plan, src, dst, core, n_nodes, chunk_rows, TA, TB):
    B = plan["B"]
    T = TA + TB
    d0 = plan["bounds"][core]
    d1 = plan["bounds"][core + 1]
    binof, slotof = plan["core_bins"][core]
    eidx = plan["order"][plan["lo"][core]:plan["hi"][core]]
    esrc = src[eidx].astype(np.int64)
    eloc = dst[eidx].astype(np.int64) - d0
    eb = binof[eloc]
    chunk = (esrc >= chunk_rows).astype(np.int64)
    o2 = np.lexsort((esrc, chunk, eb))
    eb, esrc_o, eloc_o, ch_o = eb[o2], esrc[o2], eloc[o2], chunk[o2]
    eslot = slotof[eloc_o]
    nE = len(eidx)
    key = eb * 2 + ch_o
    gstart = np.searchsorted(key, np.arange(2 * B))
    rank = np.arange(nE) - gstart[key]
    pos = np.where(ch_o == 0, rank, rank + TA * P)
    t, pp = np.divmod(pos, P)
    assert (np.where(ch_o == 0, t, 0)).max(initial=0) < TA
    assert t.max(initial=0) < T
    colpos = eb * T + t

    dstloc = np.full((P, B * T), TRASH_SLOT, np.float32)
    dstloc[pp, colpos] = eslot
    # transposed copy: per (bin,tile) column-major edge order along free dim
    dstlocT = np.ascontiguousarray(
        dstloc.T.reshape(1, B * T * P)).astype(BF)

    flat = np.zeros((B, T * P), np.int16)
    selA = ch_o == 0
    flat[eb[selA], pos[selA]] = esrc_o[selA].astype(np.int16)
    selB = ch_o == 1
    flat[eb[selB], pos[selB]] = (esrc_o[selB]
                                 - chunk_rows).astype(np.int16)
    idxAB = _wrap16(flat)

    outidx = np.full((P, B), n_nodes, np.int32)
    outidx[slotof, binof] = np.arange(d1 - d0) + d0
    return {"idxAB": idxAB, "dstloc": dstloc,
            "dstlocT": dstlocT, "outidx": outidx,
            "binof": binof, "slotof": slotof, "d0": d0, "d1": d1}


def _prepare(inputs, n_cores):
    feat = np.ascontiguousarray(np.asarray(inputs["feat"], np.float32))
    src = np.asarray(inputs["src"])
    dst = np.asarray(inputs["dst"])
    attn = np.asarray(inputs["attn"], np.float32)
    N, D = feat.shape
    H = attn.shape[0]

    WsP, WdP, bV, INVA, SCALE, ALPHA, c_of_q = _fold_weights(
        inputs["W_src"], inputs["b_src"], inputs["W_dst"], inputs["b_dst"],
        attn)
    NPAD = ((N + 2 * P - 1) // (2 * P)) * (2 * P)
    CHK = NPAD // 2
    assert CHK < 32768, "chunk too large for int16 dma_gather indices"
    plan = _build_plan(src, dst, N, n_cores, CHK)
    B = plan["B"]
    TA = TB = 0
    for c in range(n_cores):
        ca, cb = _core_chunk_counts(plan, src, dst, c, CHK)
        TA = max(TA, int(np.ceil(ca.max() / P)))
        TB = max(TB, int(np.ceil(cb.max() / P)))
    TA = ((TA + GW - 1) // GW) * GW
    TB = ((TB + GW - 1) // GW) * GW
    T = TA + TB
    VROWS = B * P

    featT = np.zeros((D, NPAD), np.float32)
    featT[:, :N] = feat.T
    featT_bf = featT.astype(BF)
    IOTA = np.ascontiguousarray(np.broadcast_to(
        np.arange(P, dtype=np.float32), (P, GW, P))).astype(BF)
    IOTAP = np.arange(P, dtype=np.float32).reshape(P, 1).astype(BF)
    ONES1 = np.ones((1, P), np.float32).astype(BF)
    IDENT = np.eye(P, dtype=np.float32).astype(BF)
    HSEL = np.zeros((P, H), np.float32)
    HSEL[np.arange(P), np.arange(P) % H] = 1.0
    HSEL = HSEL.astype(BF)

    cfg = {"N": N, "NPAD": NPAD, "B": B, "T": T, "D": D,
           "VROWS": VROWS, "H": H, "TA": TA, "TB": TB, "CHK": CHK}
    WsP_r = np.ascontiguousarray(
        WsP.reshape(2, P, D)).astype(BF)                      # [2, P, D]
    WdP_r = np.ascontiguousarray(WdP.reshape(2, P, D)).astype(BF)
    bV_r = bV.reshape(1, D).astype(BF)

    in_maps = []
    coreinfo = []
    for c in range(n_cores):
        arrs = _build_core_arrays(plan, src, dst, c, N, CHK, TA, TB)
        d0, d1 = arrs["d0"], arrs["d1"]
        binof, slotof = arrs["binof"], arrs["slotof"]
        featTd = np.zeros((D + 1, VROWS), np.float32)
        cols = binof.astype(np.int64) * P + slotof.astype(np.int64)
        featTd[:D, cols] = featT[:, d0 + np.arange(d1 - d0)]
        featTd[D, cols] = 1.0
        in_maps.append({
            "featT": featT_bf, "featTd": featTd.astype(BF),
            "WsP": WsP_r, "WdP": WdP_r, "bV": bV_r,
            "INVA": INVA, "SCALE": SCALE, "ALPHA": ALPHA,
            "IOTA": IOTA, "IOTAP": IOTAP, "ONES1": ONES1,
            "IDENT": IDENT, "HSEL": HSEL,
            "idxAB": arrs["idxAB"],
            "dstloc": arrs["dstloc"], "dstlocT": arrs["dstlocT"],
        })
        coreinfo.append((d0, d1, arrs["outidx"]))
    return cfg, in_maps, coreinfo


# ------------------------------------------------------------- bass program


def build_program(cfg, num_devices, repeat=1):
    N, NPAD, B, T = cfg["N"], cfg["NPAD"], cfg["B"], cfg["T"]
    D, VROWS, H = cfg["D"], cfg["VROWS"], cfg["H"]
    TA, TB = cfg["TA"], cfg["TB"]
    assert NPAD % P == 0

    _nq = int(os.environ.get("GAT_QUEUES", "4"))
    nc = bacc.Bacc("TRN2", target_bir_lowering=False, debug=False,
                   num_devices=num_devices,
                   num_swdge_queues=_nq,
                   dynamic_dma_scratch_size=int(
                       os.environ.get("GAT_SCRATCH", "32768")))
    if _nq > 1 and os.environ.get("GAT_NQATTR", "0") == "1":
        nc.m.attributes = (nc.m.attributes or {}) | {
            "num_swdge_queues": _nq}

    featT = nc.dram_tensor("featT", (D, NPAD), BF16, kind="ExternalInput")
    featTd = nc.dram_tensor("featTd", (D + 1, VROWS), BF16,
                            kind="ExternalInput")
    WsP = nc.dram_tensor("WsP", (2, P, D), BF16, kind="ExternalInput")
    WdP = nc.dram_tensor("WdP", (2, P, D), BF16, kind="ExternalInput")
    bV = nc.dram_tensor("bV", (1, D), BF16, kind="ExternalInput")
    INVA = None
    SCALE = nc.dram_tensor("SCALE", (P, 2), F32, kind="ExternalInput")
    ALPHA = nc.dram_tensor("ALPHA", (P, 2), F32, kind="ExternalInput")
    IOTA = nc.dram_tensor("IOTA", (P, GW * P), BF16, kind="ExternalInput")
    IOTAP = nc.dram_tensor("IOTAP", (P, 1), BF16, kind="ExternalInput")
    ONES1 = nc.dram_tensor("ONES1", (1, P), BF16, kind="ExternalInput")
    IDENT = nc.dram_tensor("IDENT", (P, P), BF16, kind="ExternalInput")
    HSEL = nc.dram_tensor("HSEL", (P, H), BF16, kind="ExternalInput")
    idxAB = nc.dram_tensor("idxAB", (P, B * T * P // 16), I16,
                           kind="ExternalInput")
    dstloc = nc.dram_tensor("dstloc", (P, B * T), F32, kind="ExternalInput")
    dstlocT = nc.dram_tensor("dstlocT", (1, B * T * P), BF16,
                             kind="ExternalInput")
    OUT = nc.dram_tensor("OUT", (B * P, D + H), BF16,
                         kind="ExternalOutput")
    Uext = [nc.dram_tensor("UextA", (NPAD // 2, D), BF16, kind="Internal"),
            nc.dram_tensor("UextB", (NPAD // 2, D), BF16, kind="Internal")]
    Vext = nc.dram_tensor("Vext", (VROWS, D), BF16, kind="Internal")

    with tile.TileContext(nc) as tc:
        _build_body(tc, nc, cfg, featT, featTd, WsP, WdP, bV, INVA, SCALE,
                    ALPHA, IOTA, IOTAP, ONES1, IDENT, HSEL, idxAB,
                    dstloc, dstlocT, OUT, Uext, Vext, repeat=repeat)
    nc.compile()
    return nc


@with_exitstack
def _build_body(ctx: ExitStack, tc, nc, cfg, featT, featTd, WsP, WdP, bV,
                INVA, SCALE, ALPHA, IOTA, IOTAP, ONES1, IDENT, HSEL,
                idxAB, dstloc, dstlocT, OUT, Uext, Vext, repeat=1):
    N, NPAD, B, T = cfg["N"], cfg["NPAD"], cfg["B"], cfg["T"]
    D, VROWS, H = cfg["D"], cfg["VROWS"], cfg["H"]
    TA, TB, CHK = cfg["TA"], cfg["TB"], cfg["CHK"]

    cpool = ctx.enter_context(tc.tile_pool(name="consts", bufs=1))

    # SWDGE queue assignment: the tile framework round-robins Pool-engine
    # DMAs onto 8 completion-sem lanes in issue order; completions within a
    # lane must stay in-order, so pin queue = issue_index % nq (8 % nq == 0
    # keeps each lane on a single queue).
    _qctr = [0]

    def next_q(nq):
        v = _qctr[0] % nq
        _qctr[0] += 1
        return v

    ws = cpool.tile([P, 2, D], BF16)
    nc.sync.dma_start(ws[:], WsP[:].rearrange("a p x -> p a x"))
    wd = cpool.tile([P, 2, D], BF16)
    nc.sync.dma_start(wd[:], WdP[:].rearrange("a p x -> p a x"))
    wdb = cpool.tile([1, D], BF16)
    nc.sync.dma_start(wdb[:], bV[:])
    scl = cpool.tile([P, 2], F32)
    nc.sync.dma_start(scl[:], SCALE[:])
    alp = cpool.tile([P, 2], F32)
    nc.sync.dma_start(alp[:], ALPHA[:])
    iota = cpool.tile([P, GW, P], BF16)
    nc.sync.dma_start(iota[:], IOTA[:].rearrange("p (g i) -> p g i", g=GW))
    iotap = cpool.tile([P, 1], BF16)
    nc.sync.dma_start(iotap[:], IOTAP[:])
    ones1 = cpool.tile([1, P], BF16)
    nc.sync.dma_start(ones1[:], ONES1[:])
    ident = cpool.tile([P, P], BF16)
    nc.sync.dma_start(ident[:], IDENT[:])
    hsel = cpool.tile([P, H], BF16)
    nc.sync.dma_start(hsel[:], HSEL[:])
    dlc = cpool.tile([P, B * T], F32)
    nc.sync.dma_start(dlc[:], dstloc[:])

    phase = os.environ.get("GAT_PHASE", "full")

    def env(k, d):
        return int(os.environ.get(k, str(d)))

    fpool = ctx.enter_context(tc.tile_pool(name="proj", bufs=3))
    upool = ctx.enter_context(tc.tile_pool(name="uout", bufs=4))
    pj = ctx.enter_context(tc.tile_pool(name="psum_proj",
                                        bufs=env("GAT_PJ", 2),
                                        space="PSUM"))
    epool = ctx.enter_context(tc.tile_pool(name="edge", bufs=2))
    spool = ctx.enter_context(tc.tile_pool(name="small", bufs=2))
    pqv = ctx.enter_context(tc.tile_pool(name="psum_qv",
                                         bufs=env("GAT_QV", 2),
                                         space="PSUM"))
    pmisc = ctx.enter_context(tc.tile_pool(name="psum_misc", bufs=1,
                                           space="PSUM"))
    pa = ctx.enter_context(tc.tile_pool(name="psum_agg",
                                        bufs=env("GAT_PAGG", 2),
                                        space="PSUM"))

    for rep in range(repeat):
      if True:

        vball = epool.tile([P, B, D], BF16, tag="vball", bufs=1)

        def projectV():
            CH = int(os.environ.get("GAT_CH", "1024"))
            nchunks = (VROWS + CH - 1) // CH
            for ci in range(nchunks):
                c0 = ci * CH
                cw = min(CH, VROWS - c0)
                nsub = cw // P
                ft = fpool.tile([P, 2, CH], BF16, tag="ft")
                nc.sync.dma_start(
                    ft[:, :, :cw],
                    featTd[0:D, :].rearrange("(a p) n -> p a n",
                                             p=P)[:, :, c0:c0 + cw])
                ftb = fpool.tile([1, CH], BF16, tag="ftb", name="ftb")
                nc.sync.dma_start(ftb[:, :cw], featTd[D:D + 1, c0:c0 + cw])
                for pi in range(0, nsub, 2):
                    pw = min(2, nsub - pi)
                    ps = pj.tile([P, 2, D], F32, tag="pv")
                    for k in range(pw):
                        si = (pi + k) * P
                        nc.tensor.matmul(ps[:, k], lhsT=ft[:, 0, si:si + P],
                                         rhs=wd[:, 0], start=True,
                                         stop=False)
                        nc.tensor.matmul(ps[:, k], lhsT=ft[:, 1, si:si + P],
                                         rhs=wd[:, 1], start=False,
                                         stop=False)
                        nc.tensor.matmul(ps[:, k], lhsT=ftb[:, si:si + P],
                                         rhs=wdb[:], start=False, stop=True)
                    if (pi // 2) % 2 == 0:
                        nc.scalar.copy(
                            vball[:, c0 // P + pi:c0 // P + pi + pw],
                            ps[:, :pw])
                    else:
                        nc.vector.tensor_copy(
                            vball[:, c0 // P + pi:c0 // P + pi + pw],
                            ps[:, :pw])

        def project(ft_dram, ncols, wtile, wbias, table):
            CH = int(os.environ.get("GAT_CH", "1024"))
            assert ncols % P == 0
            nchunks = (ncols + CH - 1) // CH
            for ci in range(nchunks):
                c0 = ci * CH
                cw = min(CH, ncols - c0)
                nsub = cw // P
                ft = fpool.tile([P, 2, CH], BF16, tag="ft")
                nc.sync.dma_start(
                    ft[:, :, :cw],
                    ft_dram[0:D, :].rearrange("(a p) n -> p a n",
                                              p=P)[:, :, c0:c0 + cw])
                if wbias is not None:
                    ftb = fpool.tile([1, CH], BF16, tag="ftb", name="ftb")
                    nc.sync.dma_start(ftb[:, :cw],
                                      ft_dram[D:D + 1, c0:c0 + cw])
                ub = upool.tile([P, CH // P, D], BF16)
                for pi in range(0, nsub, 2):
                    pw = min(2, nsub - pi)
                    ps = pj.tile([P, 2, D], F32, tag="pv")
                    for k in range(pw):
                        si = (pi + k) * P
                        nc.tensor.matmul(ps[:, k], lhsT=ft[:, 0, si:si + P],
                                         rhs=wtile[:, 0], start=True,
                                         stop=False)
                        nc.tensor.matmul(ps[:, k], lhsT=ft[:, 1, si:si + P],
                                         rhs=wtile[:, 1], start=False,
                                         stop=wbias is None)
                        if wbias is not None:
                            nc.tensor.matmul(ps[:, k],
                                             lhsT=ftb[:, si:si + P],
                                             rhs=wbias[:], start=False,
                                             stop=True)
                    if (pi // 2) % 2 == 0:
                        nc.scalar.copy(ub[:, pi:pi + pw], ps[:, :pw])
                    else:
                        nc.vector.tensor_copy(ub[:, pi:pi + pw],
                                              ps[:, :pw])
                nc.sync.dma_start(
                    table[c0:c0 + cw, :].rearrange("(s p) d -> p s d", p=P),
                    ub[:, :nsub])

        project(featT[:, 0:CHK], CHK, ws, None, Uext[0])
        projectV()
        project(featT[:, CHK:2 * CHK], CHK, ws, None, Uext[1])

      if phase == "proj":
          continue

      nq = int(os.environ.get("GAT_QUEUES", "4"))

      if phase in ("gatherbig", "gathert", "gathertbig", "gatherq"):
        for b in range(B):
          ix = spool.tile([P, T * P // 16], I16, tag="ix", bufs=3)
          nc.sync.dma_start(ix[:], idxAB[:, b * T * P // 16:
                                         (b + 1) * T * P // 16])
          sink = spool.tile([P, 1], BF16, tag="sink", name="sink")
          if phase == "gatherbig":
              ug = epool.tile([P, T, D], BF16, tag="ug", bufs=3)
              nc.gpsimd.dma_gather(ug[:, :TA], Uext[0][:],
                                   ix[:, :TA * P // 16], TA * P, TA * P, D)
              nc.vector.tensor_copy(sink[:], ug[:, 0, 0:1])
          elif phase == "gatherq":
              ug = epool.tile([P, T, D], BF16, tag="ug", bufs=3)
              TQ = T // 4
              for k in range(4):
                  nc.gpsimd.dma_gather(
                      ug[:, k * TQ:(k + 1) * TQ], Uext[0][:],
                      ix[:, k * TQ * P // 16:(k + 1) * TQ * P // 16],
                      TQ * P, TQ * P, D, queue_num=k % nq)
              nc.vector.tensor_copy(sink[:], ug[:, 0, 0:1])
          elif phase == "gathert":
              for gi in range(T // GW):
                  gt = gi * GW
                  uf = epool.tile([P, 2, GW * P], BF16, tag="uf",
                                  name="uf", bufs=5)
                  base = Uext[0][:] if gt < TA else Uext[1][:]
                  nc.gpsimd.dma_gather(
                      uf[:], base, ix[:, gt * P // 16:(gt + GW) * P // 16],
                      GW * P, GW * P, D, transpose=True,
                      queue_num=gi % nq)
              nc.vector.tensor_copy(sink[:], uf[:, 0, 0:1])
          else:
              ufb = epool.tile([P, 2, T * P // 2], BF16, tag="ufb",
                               bufs=2)
              nc.gpsimd.dma_gather(ufb[:], Uext[0][:],
                                   ix[:, :T * P // 32], T * P // 2,
                                   T * P // 2, D, transpose=True)
              nc.vector.tensor_copy(sink[:], ufb[:, 0, 0:1])
        continue

      if True:
       for b in range(B):
        ix = spool.tile([P, T * P // 16], I16, tag="ix", bufs=4)
        nc.sync.dma_start(ix[:], idxAB[:, b * T * P // 16:
                                       (b + 1) * T * P // 16])
        vb = vball[:, b]

        ug = epool.tile([P, T, D], BF16, tag="ug", bufs=4)
        TH = TA // 2
        for hf in range(2):
            nc.gpsimd.dma_gather(
                ug[:, hf * TH:(hf + 1) * TH], Uext[0][:],
                ix[:, hf * TH * P // 16:(hf + 1) * TH * P // 16],
                TH * P, TH * P, D, queue_num=next_q(nq))
        for hf in range(2):
            nc.gpsimd.dma_gather(
                ug[:, TA + hf * TH:TA + (hf + 1) * TH], Uext[1][:],
                ix[:, (TA + hf * TH) * P // 16:
                      (TA + (hf + 1) * TH) * P // 16],
                TH * P, TH * P, D, queue_num=next_q(nq))
        dlt = spool.tile([1, T * P], BF16, tag="dlt", bufs=4)
        nc.sync.dma_start(dlt[:], dstlocT[:, b * T * P:(b + 1) * T * P])

        pagg = pa.tile([P, D + H], F32, tag="pagg")
        ngroups = T // GW
        for gi in range(ngroups):
            gt = gi * GW
            col = b * T + gt
            uf = epool.tile([P, 2, GW * P], BF16, tag="uf", name="uf",
                            bufs=6)
            base = Uext[0][:] if gt < TA else Uext[1][:]
            nc.gpsimd.dma_gather(
                uf[:], base, ix[:, gt * P // 16:(gt + GW) * P // 16],
                GW * P, GW * P, D, transpose=True,
                queue_num=next_q(nq))

            s4g = epool.tile([P, GW, P], BF16, tag="s4g", name="s4g",
                             bufs=4)
            nc.vector.tensor_tensor(
                out=s4g[:], in0=iota[:],
                in1=dlc[:, col:col + GW, None].to_broadcast([P, GW, P]),
                op=mybir.AluOpType.is_equal)
            repl = pmisc.tile([P, GW * P], F32, tag="repl",
                              bufs=env("GAT_REPL", 1))
            nc.tensor.matmul(repl[:], lhsT=ones1[:],
                             rhs=dlt[:, gt * P:(gt + GW) * P],
                             start=True, stop=True)
            s4t = epool.tile([P, GW * P], BF16, tag="s4t", name="s4t",
                             bufs=4)
            nc.vector.tensor_tensor(
                out=s4t[:], in0=iotap[:].to_broadcast([P, GW * P]),
                in1=repl[:], op=mybir.AluOpType.is_equal)

            rgT = epool.tile([P, 2, GW * P], BF16, tag="rgT", name="rgT",
                             bufs=4)
            for j in range(2):
                qv = pqv.tile([P, GW * P], F32, tag="qv")
                nc.tensor.matmul(qv[:], lhsT=vb[:, j * P:(j + 1) * P],
                                 rhs=s4t[:], start=True, stop=False)
                nc.tensor.matmul(qv[:], lhsT=ident[:], rhs=uf[:, j],
                                 start=False, stop=True)
                nc.scalar.activation(rgT[:, j], qv[:],
                                     mybir.ActivationFunctionType.Prelu,
                                     scale=scl[:, j:j + 1],
                                     alpha=alp[:, j:j + 1])

            psc = pmisc.tile([P, GW, H], F32, tag="psc",
                             bufs=env("GAT_PSC", 1))
            for t in range(GW):
                nc.tensor.matmul(psc[:, t],
                                 lhsT=rgT[:, 0, t * P:(t + 1) * P],
                                 rhs=hsel[:], start=True, stop=False)
                nc.tensor.matmul(psc[:, t],
                                 lhsT=rgT[:, 1, t * P:(t + 1) * P],
                                 rhs=hsel[:], start=False, stop=True)

            wg = epool.tile([P, GW, D + H], BF16, tag="wg", name="wg",
                            bufs=4)
            expv = wg[:, :, D:D + H]
            nc.scalar.activation(expv, psc[:],
                                 mybir.ActivationFunctionType.Exp)
            nc.vector.tensor_tensor(
                out=wg[:, :, :D].rearrange("p g (c h) -> p g c h", h=H),
                in0=ug[:, gt:gt + GW].rearrange("p g (c h) -> p g c h", h=H),
                in1=wg[:, :, None, D:D + H].to_broadcast([P, GW, D // H, H]),
                op=mybir.AluOpType.mult)
            for t in range(GW):
                nc.tensor.matmul(pagg[:], lhsT=s4g[:, t], rhs=wg[:, t],
                                 start=(gi == 0 and t == 0),
                                 stop=(gi == ngroups - 1 and t == GW - 1))

        o2 = spool.tile([P, D + H], BF16, tag="o2")
        nc.scalar.copy(o2[:], pagg[:])
        nc.sync.dma_start(OUT[b * P:(b + 1) * P, :], o2[:])


# ------------------------------------------------------------------ entry

_LAST_RESULTS = {}


def kernel(**inputs):
    import time as _time
    t0 = _time.time()
    cfg, in_maps, coreinfo = _prepare(inputs, N_CORES)
    _LAST_RESULTS["inva_row"] = np.asarray(in_maps[0]["INVA"][0],
                                           np.float32)
    t1 = _time.time()
    nc = build_program(cfg, N_CORES,
                       repeat=int(os.environ.get("GAT_REPEAT", "1")))
    t2 = _time.time()
    res = bass_utils.run_bass_kernel_spmd(
        nc, in_maps, core_ids=list(range(N_CORES)))
    t3 = _time.time()
    print(f"[kernel] prep {t1-t0:.1f}s build+compile {t2-t1:.1f}s "
          f"run {t3-t2:.1f}s", file=sys.stderr)
    _LAST_RESULTS["res"] = res
    _LAST_RESULTS["cfg"] = cfg
    N, D, H = cfg["N"], cfg["D"], cfg["H"]
    DO = D // H
    inva_row = _LAST_RESULTS.get("inva_row")
    out = np.zeros((N, D), np.float32)
    for c, (d0, d1, outidx) in enumerate(coreinfo):
        gg = outidx.T.reshape(-1)
        mask = gg < N
        raw = np.asarray(res.results[c]["OUT"], np.float32)
        den = raw[:, D:D + H]
        den = np.where(den == 0, 1.0, den)
        norm = (raw[:, :D].reshape(-1, DO, H) / den[:, None, :]
                ).reshape(-1, D) * inva_row[None, :]
        out[gg[mask]] = norm[mask]
    # newcol q -> (h = q % H, d = q // H); undo on host + u-side bias
    out = out.reshape(N, DO, H).transpose(0, 2, 1)
    b_src = np.asarray(inputs["b_src"], np.float32)
    if np.any(b_src):
        out = out + b_src.reshape(1, H, DO)
    return np.ascontiguousarray(out)

